# revision 10
# baseline (speedup 1.0000x reference)
"""Diagonal SSM kernel for 8 Trainium2 NeuronCores.

Math (per batch element b, sharded one per core):
    alpha = sigmoid(u @ Wa.T + ba)          (S, N)
    Bu    = u @ Wb.T + bb                   (S, N)
    x_t   = alpha_t * x_{t-1} + Bu_t        (scan over S)
    y     = xs @ C.T + u @ Dm.T             (S, D)

Device strategy (per core):
  - u (S, D) fp32 is DMA'd in naturally (HWDGE plain copies), downcast to
    bf16 on DVE/ACT, transposed on the TensorEngine (bf16 transpose-mode
    matmul against an identity, 1 cyc/row) into PSUM, and copied out both
    as bf16 uT [D x S] tiles (for GEMM-B) and as an fp8e4 copy (for
    GEMM-A). No SWDGE, no DRAM scratch: SWDGE cast-DMAs starve the HWDGE
    queues on the shared SDMA engines.
  - GEMM-A in fp8 DoubleRow (2 k-tiles per matmul, half the PE cycles):
    psum[n, s-chunk] = sum_d Wab8T[d, n-tile] . uT8[d, s-chunk], with the
    weights pre-scaled by 64 on host (keeps them out of the fp8 subnormal
    range) and the 1/64 rescale folded into the ScalarE activation that
    applies sigmoid(+ba) / identity(+bb) straight out of PSUM. The
    alpha/Bu path only contributes ~3% of the output magnitude, so fp8
    error here is strongly attenuated.
  - Recurrence: native VectorE tensor_tensor_scan (op0=mult, op1=add,
    fp32 internal state) along the free dim, chunk-chained via a
    per-partition initial value.
  - GEMM-B (bf16 - it dominates the output magnitude): y[s-tile, d] =
    xsT.T @ CT + uT.T @ DmT accumulated in PSUM, copied to SBUF
    (DVE/ACT alternating) and DMA'd out as fp32.
  - Emission is software-pipelined: ingest for chunk sc+2 and GEMM-B for
    chunk sc-1 are emitted around GEMM-A(sc) so the PE queue never heads
    into a matmul whose scan dependency hasn't cleared yet.

Params are pre-packed on host (transposed, fp8/bf16) - standard weight
packing. The full u tensor is read on device in fp32.
"""

import numpy as np
import ml_dtypes

B, S, D, N = 8, 4096, 1024, 256
NCORES = 8
KT = D // 128          # 8 contraction tiles
SC = 512               # s-chunk (matmul free dim / PSUM bank / ingest chunk)
NSC = S // SC          # 8 s-chunks
WAB_SCALE = 64.0       # fp8 weight pre-scale for GEMM-A

_CACHE = {}
LAST_RESULTS = None    # test harness reads profiling info from here


def _build_program():
    import concourse.mybir as mybir
    import concourse.tile as tile
    from concourse import bacc
    from concourse.masks import make_identity

    fp32 = mybir.dt.float32
    bf16 = mybir.dt.bfloat16
    fp8 = mybir.dt.float8e4
    AF = mybir.ActivationFunctionType
    OP = mybir.AluOpType
    DR = mybir.MatmulPerfMode.DoubleRow

    nc = bacc.Bacc(
        "TRN2",
        target_bir_lowering=False,
        debug=False,
        enable_asserts=False,
        num_devices=NCORES,
    )

    u = nc.dram_tensor("u", [S, D], fp32, kind="ExternalInput").ap()
    wab8 = nc.dram_tensor("wab8", [128, KT, 2 * N], fp8, kind="ExternalInput").ap()
    bias = nc.dram_tensor("bias", [128, 4], fp32, kind="ExternalInput").ap()
    ct8 = nc.dram_tensor("ct8", [128, 2, D], fp8, kind="ExternalInput").ap()
    dmt = nc.dram_tensor("dmt", [D, D], bf16, kind="ExternalInput").ap()
    y = nc.dram_tensor("y", [S, D], fp32, kind="ExternalOutput").ap()

    with tile.TileContext(nc) as tc:
        with (
            tc.tile_pool(name="consts", bufs=1) as consts,
            tc.tile_pool(name="data", bufs=1) as data,
            tc.tile_pool(name="unat", bufs=4) as unat,
            tc.tile_pool(name="ab", bufs=3) as abpool,
            tc.tile_pool(name="xs", bufs=3) as xspool,
            tc.tile_pool(name="psA", bufs=2, space="PSUM") as psA,
            tc.tile_pool(name="psB", bufs=3, space="PSUM") as psB,
            tc.tile_pool(name="psT", bufs=3, space="PSUM") as psT,
            tc.tile_pool(name="ypool", bufs=3) as ypool,
        ):
            # ---- param tiles (loads emitted in startup-criticality order below) ----
            wab8_sb = consts.tile([128, KT, 2 * N], fp8, name="wab8_sb")
            ct8_sb = consts.tile([128, 2, D], fp8, name="ct8_sb")
            dmt_sb = [consts.tile([128, D], bf16, name=f"dmt{k}") for k in range(KT)]
            bias_sb = consts.tile([128, 4], fp32, name="bias_sb")
            ident16_sb = consts.tile([128, 128], bf16, name="ident16_sb")

            def load_params_early():
                # built on the (otherwise idle) GpSimd engine: no DMA in the
                # critical startup chain
                make_identity(nc, ident16_sb[:])

            def warm_pe(n):
                """Junk identity transposes to ramp the PE out of its low
                p-state while the first u tiles are still in flight. Results
                are never read; WAW on the rotating psT bufs keeps them
                back-to-back."""
                for _ in range(n):
                    ps = psT.tile([128, SC], bf16, name="pst", tag="pst")
                    nc.tensor.transpose(
                        ps[:, 0:128], ident16_sb[:], ident16_sb[:]
                    )

            def load_params_mid():
                # needed by gemm_a(0): weights + activation biases
                nc.sync.dma_start(out=wab8_sb[:], in_=wab8[:])
                nc.sync.dma_start(out=bias_sb[:], in_=bias[:])

            def load_params_late():
                # needed by gemm_b(0), which runs after gemm_a(1)
                nc.sync.dma_start(out=ct8_sb[:], in_=ct8[:])
                for k in range(KT):
                    nc.sync.dma_start(out=dmt_sb[k][:], in_=dmt[k * 128:(k + 1) * 128, :])

            uT = [data.tile([128, S], bf16, name=f"uT{k}") for k in range(KT)]
            uT8 = data.tile([128, KT, S], fp8, name="uT8")

            def ingest_dma(sc, split=False):
                """Load 4 s-tiles of u (fp32) and downcast to bf16. Returns
                the bf16 tiles for ingest_tp. split=True emits all the first
                d-halves before the second so the k<4 transposes can start
                after half the chunk has landed (startup)."""
                ut_tiles = []
                uns = []
                for t in range(4):
                    st = sc * 4 + t
                    un = unat.tile([128, D], fp32, name="unat", tag="unat")
                    un16 = unat.tile([128, D], bf16, name="un16", tag="un16", bufs=8)
                    uns.append(un)
                    ut_tiles.append(un16)
                halves = (0, 1) if split else (None,)
                for h in halves:
                    dsl = slice(0, D) if h is None else slice(h * (D // 2),
                                                              (h + 1) * (D // 2))
                    for t in range(4):
                        st = sc * 4 + t
                        nc.sync.dma_start(out=uns[t][:, dsl],
                                          in_=u[st * 128:(st + 1) * 128, dsl])
                        nc.vector.tensor_copy(ut_tiles[t][:, dsl], uns[t][:, dsl])
                return ut_tiles

            def ingest_tp(sc, ut_tiles):
                """PE-transpose each 128x128 block into PSUM, copy into uT
                (bf16) and uT8 (fp8, for the DoubleRow GEMM-A)."""
                ssl = slice(sc * SC, (sc + 1) * SC)
                for k in range(KT):
                    ps = psT.tile([128, SC], bf16, name="pst", tag="pst")
                    for t in range(4):
                        nc.tensor.transpose(
                            ps[:, t * 128:(t + 1) * 128],
                            ut_tiles[t][:, k * 128:(k + 1) * 128],
                            ident16_sb[:],
                        )
                    nc.scalar.copy(uT[k][:, ssl], ps[:])
                    nc.vector.tensor_copy(uT8[:, k, ssl], ps[:])

            def gemm_a(sc, ab_tiles=None, hsl=None):
                """fp8 DoubleRow GEMM for alpha/Bu; the 1/WAB_SCALE rescale is
                folded into the ScalarE activation. hsl selects a sub-range of
                the chunk (for the fine-grained last chunk)."""
                if ab_tiles is None:
                    ab_tiles = [abpool.tile([128, SC], bf16, name=f"ab{nt}",
                                            tag=f"ab{nt}") for nt in range(4)]
                hsl = hsl if hsl is not None else slice(0, SC)
                sw = hsl.stop - hsl.start
                ssl = slice(sc * SC + hsl.start, sc * SC + hsl.stop)
                for nt in range(4):
                    ps = psA.tile([128, SC], fp32, name="psa", tag="psa")
                    for kp in range(KT // 2):
                        nc.tensor.matmul(
                            ps[:, 0:sw],
                            wab8_sb[:, 2 * kp:2 * kp + 2, nt * 128:(nt + 1) * 128],
                            uT8[:, 2 * kp:2 * kp + 2, ssl],
                            start=(kp == 0),
                            stop=(kp == KT // 2 - 1),
                            perf_mode=DR,
                        )
                    nc.scalar.activation(
                        ab_tiles[nt][:, hsl], ps[:, 0:sw],
                        AF.Sigmoid if nt < 2 else AF.Identity,
                        bias=bias_sb[:, nt:nt + 1],
                        scale=1.0 / WAB_SCALE,
                    )
                return ab_tiles

            def alloc_xs():
                xs8 = xspool.tile([128, 2, SC], fp8, name="xs8", tag="xs8")
                xs_tiles = [xspool.tile([128, SC], bf16, name=f"xs{h}",
                                        tag=f"xs{h}") for h in range(2)]
                return xs_tiles, xs8

            def scan(ab_tiles, prev_xs, xs_pack=None, hsl=None):
                """Chunk-chained native scan; also makes the fp8 DR-packed
                copy for GEMM-B's C part. hsl chains within the chunk."""
                xs_tiles, xs8 = xs_pack if xs_pack is not None else alloc_xs()
                hsl = hsl if hsl is not None else slice(0, SC)
                for h in range(2):
                    o = xs_tiles[h]
                    if hsl.start == 0:
                        init = 0.0 if prev_xs is None else prev_xs[0][h][:, SC - 1:SC]
                    else:
                        init = o[:, hsl.start - 1:hsl.start]
                    nc.vector.tensor_tensor_scan(
                        o[:, hsl],
                        ab_tiles[h][:, hsl],
                        ab_tiles[2 + h][:, hsl],
                        init,
                        op0=OP.mult,
                        op1=OP.add,
                    )
                    nc.vector.tensor_copy(xs8[:, h, hsl], o[:, hsl])
                return xs_tiles, xs8

            def gemm_b(sc, xs_pack, trange=range(4)):
                # psum holds 64x the true output (ct8/dmt are pre-scaled by
                # 64 on host so the fp8 C tiles stay in e4m3 normal range);
                # the 1/64 rescale rides the psum->sbuf copy, alternating
                # ACT/DVE so neither engine becomes critical.
                xs_tiles, xs8 = xs_pack
                for t in trange:
                    st = sc * 4 + t
                    stsl = slice(st * 128, (st + 1) * 128)
                    tsl = slice(t * 128, (t + 1) * 128)
                    ytile = ypool.tile([128, D], fp32, name="ytile", tag="ytile")
                    # k-major over both d-bank psum groups (and the C matmul
                    # last) so gemm_b(0) streams with the dmt DMA instead of
                    # waiting for the full weight load; each uT stationary is
                    # also reused across both banks.
                    pss = [psB.tile([128, SC], fp32, name="psb", tag="psb")
                           for _ in range(2)]
                    dsls = [slice(dc * SC, (dc + 1) * SC) for dc in range(2)]
                    for k in range(KT):
                        for dc in range(2):
                            nc.tensor.matmul(pss[dc][:], uT[k][:, stsl],
                                             dmt_sb[k][:, dsls[dc]],
                                             start=(k == 0), stop=False)
                    for dc in range(2):
                        nc.tensor.matmul(pss[dc][:], xs8[:, :, tsl],
                                         ct8_sb[:, :, dsls[dc]],
                                         start=False, stop=True, perf_mode=DR)
                        if dc == 0:
                            nc.scalar.activation(ytile[:, dsls[dc]], pss[dc][:],
                                                 AF.Identity, scale=1.0 / WAB_SCALE)
                        else:
                            nc.vector.tensor_scalar_mul(ytile[:, dsls[dc]], pss[dc][:],
                                                        1.0 / WAB_SCALE)
                    nc.sync.dma_start(out=y[stsl, :], in_=ytile[:])

            # ---- software-pipelined emission ----
            # PE order: W T0 A0 T1 | A1 s1 B0 T2 | A2 s2 B1 T3 | ... so the
            # PE never queues a transpose ahead of GEMM work whose data is
            # already resident, and param DMAs land before first use. The
            # last chunk runs in quarters so only ~1/4 of its GEMM-B remains
            # un-overlapped at the end.
            load_params_early()
            warm_pe(40)
            c0 = ingest_dma(0, split=True)
            load_params_mid()
            ingest_tp(0, c0)
            ab0 = gemm_a(0)
            c1 = ingest_dma(1)
            load_params_late()
            xs_prev = scan(ab0, None)
            ingest_tp(1, c1)
            pending = ingest_dma(2)
            for sc in range(1, NSC):
                ab = gemm_a(sc)
                xs_cur = scan(ab, xs_prev)
                gemm_b(sc - 1, xs_prev)
                cnext = ingest_dma(sc + 2) if sc + 2 < NSC else None
                if sc + 1 < NSC:
                    ingest_tp(sc + 1, pending)
                pending = cnext
                xs_prev = xs_cur
            gemm_b(NSC - 1, xs_prev)

    nc.compile()
    return nc


def _get_program():
    if "nc" not in _CACHE:
        _CACHE["nc"] = _build_program()
    return _CACHE["nc"]


def kernel(u, Wa, ba, Wb, bb, C, Dm):
    global LAST_RESULTS
    from concourse.bass_utils import run_bass_kernel_spmd

    nc = _get_program()

    u = np.asarray(u, dtype=np.float32)
    bf = ml_dtypes.bfloat16
    f8 = ml_dtypes.float8_e4m3
    wab = np.concatenate([np.asarray(Wa), np.asarray(Wb)], axis=0).T   # (D, 2N)
    wab8_np = np.ascontiguousarray(
        (np.asarray(wab, np.float32) * WAB_SCALE)
        .reshape(KT, 128, 2 * N).transpose(1, 0, 2)
    ).astype(f8)                                                       # (128, KT, 2N)
    bias_np = np.ascontiguousarray(
        np.concatenate([np.asarray(ba), np.asarray(bb)]).astype(np.float32)
        .reshape(4, 128).T
    )                                                                  # (128, 4)
    # C.T scaled by 64 (keeps fp8 e4m3 out of the subnormal range), packed
    # [n-within-half, half, d] for the DoubleRow C matmul; Dm.T carries the
    # same 64x so both GEMM-B paths share one psum scale.
    ct8_np = np.ascontiguousarray(
        (np.asarray(C, np.float32).T * WAB_SCALE)
        .reshape(2, 128, D).transpose(1, 0, 2)
    ).astype(f8)                                                       # (128, 2, D)
    dmt_np = np.ascontiguousarray(
        np.asarray(Dm, np.float32).T * WAB_SCALE
    ).astype(bf)                                                       # (D, D)

    in_maps = [
        {
            "u": np.ascontiguousarray(u[b]),
            "wab8": wab8_np,
            "bias": bias_np,
            "ct8": ct8_np,
            "dmt": dmt_np,
        }
        for b in range(B)
    ]

    res = run_bass_kernel_spmd(nc, in_maps, core_ids=list(range(NCORES)))
    LAST_RESULTS = res
    return np.stack([r["y"] for r in res.results], axis=0)

